# revision 1
# baseline (speedup 1.0000x reference)
"""Multi-head attention (B=2, S=2048, d_model=768, H=12) on 8 TRN2 NeuronCores.

Sharding: 2-way data parallel over batch x 4-way tensor parallel over heads
(3 heads / 192-wide d_model slice per core). Host compacts masked keys away
(gather of unmasked key/value rows), pads to a 128 multiple, and passes a 0/1
validity vector; softmax needs no mask handling on device (pad keys get V=0
and a 0 in the denominator ones-column). Per core:

    Q^T [192,2048], K^T [192,KP] via projections (dq on partitions)
    V   [KP,192] natural layout, x3 per-head [V_h | valid] blocks
    per head: scores^T[k,q] = K_h^T.T @ Q_h^T ; es = exp(s/8) on ACT
              ctx'^T[65,q] += [V_h|valid].T @ es  (row 64 = denominator)
              ctx = ctx * recip(denom) (DVE + gpsimd partition_broadcast)
    out_partial[2048,768] = ctx^T.T @ Wo_g, summed on host + bo.

All matmul operands are bf16 (PSUM accumulation f32); heads 0/1 issue score
matmuls into disjoint PE row groups back-to-back (2x concurrency) and share
one PSUM scores tile so a single ACT exp covers both heads. The output
projection is interleaved per query chunk to keep the PE warm.
"""

import math
import os

import numpy as np

B = 2
S = 2048
DM = 768
H = 12
DH = 64
G = 4              # head-group (tensor-parallel) degree
HPG = H // G       # heads per core
DQ = HPG * DH      # 192 d_model slice per core
NCORES = 8
P = 128

_prog_cache = {}


def _chunks(total, step):
    out = []
    o = 0
    while o < total:
        w = min(step, total - o)
        out.append((o, w))
        o += w
    return out


def _groups(n, g):
    out = []
    o = 0
    while o < n:
        out.append(list(range(o, min(o + g, n))))
        o += g
    return out


def _build_nc(KP):
    import concourse.bass as bass
    import concourse.mybir as mybir
    import concourse.tile as tile
    from concourse import bacc

    F32 = mybir.dt.float32
    BF = mybir.dt.bfloat16
    AFT = mybir.ActivationFunctionType

    T = KP // P            # key tiles
    NKT = DM // P          # 6 contraction tiles for projections
    QCH = _chunks(S, 512)
    KCH = _chunks(KP, 512)
    NCH = _chunks(DM, 512)

    nc = bacc.Bacc(None, target_bir_lowering=False)
    xqT = nc.declare_dram_parameter("xqT", [DM, S], BF, isOutput=False)
    xkT = nc.declare_dram_parameter("xkT", [DM, KP], BF, isOutput=False)
    xvT = nc.declare_dram_parameter("xvT", [DM, KP], BF, isOutput=False)
    wq = nc.declare_dram_parameter("wq", [DM, DQ], BF, isOutput=False)
    wk = nc.declare_dram_parameter("wk", [DM, DQ], BF, isOutput=False)
    wv = nc.declare_dram_parameter("wv", [DM, DQ], BF, isOutput=False)
    wo = nc.declare_dram_parameter("wo", [DQ, DM], BF, isOutput=False)
    bq = nc.declare_dram_parameter("bq", [DQ, 1], F32, isOutput=False)
    bk = nc.declare_dram_parameter("bk", [DQ, 1], F32, isOutput=False)
    bv = nc.declare_dram_parameter("bv", [1, DQ], F32, isOutput=False)
    vm = nc.declare_dram_parameter("vm", [P, T], F32, isOutput=False)
    out = nc.declare_dram_parameter("out", [S, DM], F32, isOutput=True)

    with tile.TileContext(nc) as tc:
        with (
            tc.tile_pool(name="persist", bufs=1) as persist,
            tc.tile_pool(name="acts", bufs=18) as acts,
            tc.tile_pool(name="es", bufs=4) as espool,
            tc.tile_pool(name="norm", bufs=6) as norm,
            tc.tile_pool(name="osb", bufs=4) as osb,
        ):
            # ---- weights / constants ----
            WQ = persist.tile([P, NKT, DQ], BF, tag="WQ")
            WK = persist.tile([P, NKT, DQ], BF, tag="WK")
            WV = persist.tile([P, NKT, DQ], BF, tag="WV")
            nc.sync.dma_start(out=WQ, in_=wq[:, :].rearrange("(kt p) m -> p kt m", p=P))
            nc.sync.dma_start(out=WK, in_=wk[:, :].rearrange("(kt p) m -> p kt m", p=P))
            nc.sync.dma_start(out=WV, in_=wv[:, :].rearrange("(kt p) m -> p kt m", p=P))
            WO0 = persist.tile([P, DM], BF, tag="WO0")   # wo rows 0:128 (h0,h1)
            WO2 = persist.tile([DH, DM], BF, tag="WO2")  # wo rows 128:192 (h2)
            nc.sync.dma_start(out=WO0, in_=wo[0:P, :])
            nc.sync.dma_start(out=WO2, in_=wo[P:DQ, :])
            BQ0 = persist.tile([P, 1], F32, tag="BQ0")
            BQ1 = persist.tile([DH, 1], F32, tag="BQ1")
            BK0 = persist.tile([P, 1], F32, tag="BK0")
            BK1 = persist.tile([DH, 1], F32, tag="BK1")
            nc.sync.dma_start(out=BQ0, in_=bq[0:P, :])
            nc.sync.dma_start(out=BQ1, in_=bq[P:DQ, :])
            nc.sync.dma_start(out=BK0, in_=bk[0:P, :])
            nc.sync.dma_start(out=BK1, in_=bk[P:DQ, :])
            BV = persist.tile([P, DQ], F32, tag="BV")
            nc.sync.dma_start(out=BV, in_=bv[:, :].to_broadcast([P, DQ]))
            VM = persist.tile([P, T], F32, tag="VM")
            nc.sync.dma_start(out=VM, in_=vm[:, :])

            # ---- persistent activations ----
            QT0 = persist.tile([P, S], BF, tag="QT0")    # heads 0,1
            QT1 = persist.tile([DH, S], BF, tag="QT1")   # head 2
            KT0 = persist.tile([P, KP], BF, tag="KT0")
            KT1 = persist.tile([DH, KP], BF, tag="KT1")
            # V blocks padded to 128 cols (cols 0:64 V, 64 ones, 65:128 zero)
            # so PV ldweights gets FWL (needs exactly 128 weight columns)
            VP = persist.tile([P, T, HPG * P], BF, tag="VP")
            nc.vector.memset(VP, 0.0)
            CTX01 = persist.tile([P, S], BF, tag="CTX01")  # h0 rows 0:64, h1 64:128
            CTX2 = persist.tile([DH, S], BF, tag="CTX2")

            # One PSUM pool set for the whole kernel (no pool-scope barriers):
            # big: 2-bank slots x2 (scores double-buffer + O-proj psum)
            # sml: 1-bank slots x3 (projection psums + ctx accumulators)
            _big_cm = tc.tile_pool(name="big_ps", bufs=2, space="PSUM")
            _sml_cm = tc.tile_pool(name="sml_ps", bufs=2, space="PSUM")
            _opo_cm = tc.tile_pool(name="opo_ps", bufs=1, space="PSUM")
            big_ps = _big_cm.__enter__()
            sml_ps = _sml_cm.__enter__()
            opo_ps = _opo_cm.__enter__()
            ctx_stack = [_big_cm, _sml_cm, _opo_cm]

            # ---- phase A: projections (KT -> QT -> VP) ----
            XK = []
            for kt in range(NKT):
                xt = acts.tile([P, S], BF, tag="xact", name=f"xk{kt}")
                nc.sync.dma_start(out=xt[:, 0:KP], in_=xkT[kt * P:(kt + 1) * P, :])
                XK.append(xt)
            for (c0, cw) in KCH:
                for m, (dst, bias, mw) in enumerate(
                    [(KT0, BK0, P), (KT1, BK1, DH)]
                ):
                    ps = sml_ps.tile([mw, 512], F32, tag="sml", name=f"kps{c0}_{m}")
                    for kt in range(NKT):
                        nc.tensor.matmul(
                            ps[:, 0:cw],
                            lhsT=WK[:, kt, m * P:m * P + mw],
                            rhs=XK[kt][:, c0:c0 + cw],
                            start=(kt == 0),
                            stop=(kt == NKT - 1),
                        )
                    nc.scalar.activation(
                        dst[:, c0:c0 + cw], ps[:, 0:cw], AFT.Identity, bias=bias
                    )
            XQ = []
            for kt in range(NKT):
                xt = acts.tile([P, S], BF, tag="xact", name=f"xq{kt}")
                nc.sync.dma_start(out=xt, in_=xqT[kt * P:(kt + 1) * P, :])
                XQ.append(xt)
            for (c0, cw) in QCH:
                for m, (dst, bias, mw) in enumerate(
                    [(QT0, BQ0, P), (QT1, BQ1, DH)]
                ):
                    ps = sml_ps.tile([mw, 512], F32, tag="sml", name=f"qps{c0}_{m}")
                    for kt in range(NKT):
                        nc.tensor.matmul(
                            ps[:, 0:cw],
                            lhsT=WQ[:, kt, m * P:m * P + mw],
                            rhs=XQ[kt][:, c0:c0 + cw],
                            start=(kt == 0),
                            stop=(kt == NKT - 1),
                        )
                    nc.scalar.activation(
                        dst[:, c0:c0 + cw], ps[:, 0:cw], AFT.Identity, bias=bias
                    )
            XV = []
            for kt in range(NKT):
                xt = acts.tile([P, S], BF, tag="xact", name=f"xv{kt}")
                nc.sync.dma_start(out=xt[:, 0:KP], in_=xvT[kt * P:(kt + 1) * P, :])
                XV.append(xt)
            for t in range(T):
                ps = sml_ps.tile([P, DQ], F32, tag="sml", name=f"vps{t}")
                for kt in range(NKT):
                    nc.tensor.matmul(
                        ps,
                        lhsT=XV[kt][:, t * P:(t + 1) * P],
                        rhs=WV[:, kt, :],
                        start=(kt == 0),
                        stop=(kt == NKT - 1),
                    )
                vview = VP[:, t, :].rearrange("p (h c) -> p h c", c=P)
                nc.vector.tensor_add(
                    vview[:, :, 0:DH],
                    ps.rearrange("p (h d) -> p h d", d=DH),
                    BV[:, :].rearrange("p (h d) -> p h d", d=DH),
                )
                nc.vector.tensor_scalar_mul(
                    vview[:, :, 0:DH], vview[:, :, 0:DH], VM[:, t:t + 1]
                )
                nc.vector.tensor_copy(
                    vview[:, :, DH:DH + 1],
                    VM[:, t:t + 1].to_broadcast([P, HPG, 1]),
                )

            # ---- phase B+C: attention + output projection, per query chunk ----
            def attn_norm(ctx, dst, c0, cw, uid):
                # evict PSUM on ACT (frees the ctx bank + off DVE critical
                # path), then normalize from SBUF
                cs = norm.tile([DH, 512], F32, tag="cs", name=f"cs{uid}")
                nc.scalar.activation(cs[:, 0:cw], ctx[0:DH, 0:cw], AFT.Identity)
                dn = norm.tile([1, 512], F32, tag="dn", name=f"dn{uid}")
                nc.vector.tensor_copy(dn[:, 0:cw], ctx[DH:DH + 1, 0:cw])
                rc = norm.tile([1, 512], F32, tag="rc", name=f"rc{uid}")
                nc.vector.reciprocal_approx_fast(rc[:, 0:cw], dn[:, 0:cw])
                bc = norm.tile([DH, 512], F32, tag="bc", name=f"bc{uid}")
                nc.gpsimd.partition_broadcast(bc[:, 0:cw], rc[:, 0:cw])
                nc.vector.tensor_mul(dst[:, c0:c0 + cw], cs[:, 0:cw], bc[:, 0:cw])

            for ci, (c0, cw) in enumerate(QCH):
                # heads 0+1 paired: adjacent matmuls into disjoint PE row
                # groups (2x concurrency), one shared scores tile + exp per t;
                # scores double-buffered so PE streams through ACT exps.
                ctx0 = sml_ps.tile([P, 512], F32, tag="sml", name=f"c0_{ci}")
                ctx1 = sml_ps.tile([P, 512], F32, tag="sml", name=f"c1_{ci}")
                for t in range(T):
                    sp = big_ps.tile([P, 2 * 512], F32, tag="big", name=f"sp{ci}_{t}")
                    nc.tensor.matmul(
                        sp[:, 0:cw],
                        lhsT=KT0[0:DH, t * P:(t + 1) * P],
                        rhs=QT0[0:DH, c0:c0 + cw],
                        start=True, stop=True,
                    )
                    nc.tensor.matmul(
                        sp[:, 512:512 + cw],
                        lhsT=KT0[DH:P, t * P:(t + 1) * P],
                        rhs=QT0[DH:P, c0:c0 + cw],
                        start=True, stop=True,
                    )
                    es = espool.tile([P, 2 * 512], BF, tag="es", name=f"es{ci}_{t}")
                    nc.scalar.activation(
                        es, sp, AFT.Exp, bias=0.0, scale=1.0 / math.sqrt(DH),
                    )
                    nc.tensor.matmul(
                        ctx0[:, 0:cw],
                        lhsT=VP[:, t, 0:P],
                        rhs=es[:, 0:cw],
                        start=(t == 0), stop=(t == T - 1),
                    )
                    nc.tensor.matmul(
                        ctx1[:, 0:cw],
                        lhsT=VP[:, t, P:2 * P],
                        rhs=es[:, 512:512 + cw],
                        start=(t == 0), stop=(t == T - 1),
                    )
                attn_norm(ctx0, CTX01[0:DH, :], c0, cw, f"a{ci}")
                attn_norm(ctx1, CTX01[DH:P, :], c0, cw, f"b{ci}")
                # head 2: two key tiles per scores buffer
                ctx2 = sml_ps.tile([P, 512], F32, tag="sml", name=f"c2_{ci}")
                for tg in _groups(T, 2):
                    ln = len(tg)
                    sp = big_ps.tile([P, 2 * 512], F32, tag="big", name=f"sp2_{ci}_{tg[0]}")
                    for i, t in enumerate(tg):
                        nc.tensor.matmul(
                            sp[:, i * 512:i * 512 + cw],
                            lhsT=KT1[0:DH, t * P:(t + 1) * P],
                            rhs=QT1[0:DH, c0:c0 + cw],
                            start=True, stop=True,
                        )
                    es = espool.tile([P, 2 * 512], BF, tag="es", name=f"es2_{ci}_{tg[0]}")
                    nc.scalar.activation(
                        es[:, 0:ln * 512], sp[:, 0:ln * 512],
                        AFT.Exp, bias=0.0, scale=1.0 / math.sqrt(DH),
                    )
                    for i, t in enumerate(tg):
                        nc.tensor.matmul(
                            ctx2[:, 0:cw],
                            lhsT=VP[:, t, 2 * P:3 * P],
                            rhs=es[:, i * 512:i * 512 + cw],
                            start=(t == 0), stop=(t == T - 1),
                        )
                attn_norm(ctx2, CTX2, c0, cw, f"c{ci}")
                # output projection for this chunk's query tiles
                for mi in range(cw // P):
                    m = c0 // P + mi
                    po = opo_ps.tile([P, DM], F32, tag="opo", name=f"po{m}")
                    for (n0, nw) in NCH:
                        nc.tensor.matmul(
                            po[:, n0:n0 + nw],
                            lhsT=CTX01[:, m * P:(m + 1) * P],
                            rhs=WO0[:, n0:n0 + nw],
                            start=True, stop=False,
                        )
                        nc.tensor.matmul(
                            po[:, n0:n0 + nw],
                            lhsT=CTX2[:, m * P:(m + 1) * P],
                            rhs=WO2[:, n0:n0 + nw],
                            start=False, stop=True,
                        )
                    po_sb = osb.tile([P, DM], F32, tag="posb", name=f"posb{m}")
                    nc.vector.tensor_copy(po_sb, po)
                    nc.sync.dma_start(out=out[m * P:(m + 1) * P, :], in_=po_sb)

            for _p in reversed(ctx_stack):
                _p.__exit__(None, None, None)
    nc.compile()
    return nc


def _get_prog(KP):
    if KP not in _prog_cache:
        _prog_cache[KP] = _build_nc(KP)
    return _prog_cache[KP]


def _run(inputs, trace=False):
    import ml_dtypes
    from concourse.bass_utils import run_bass_kernel_spmd

    BF = ml_dtypes.bfloat16

    query = np.asarray(inputs["query"], dtype=np.float32)
    key = np.asarray(inputs["key"], dtype=np.float32)
    value = np.asarray(inputs["value"], dtype=np.float32)
    mask = np.asarray(inputs["mask"])
    Wq = np.asarray(inputs["Wq"], dtype=np.float32)
    bq = np.asarray(inputs["bq"], dtype=np.float32)
    Wk = np.asarray(inputs["Wk"], dtype=np.float32)
    bk = np.asarray(inputs["bk"], dtype=np.float32)
    Wv = np.asarray(inputs["Wv"], dtype=np.float32)
    bv = np.asarray(inputs["bv"], dtype=np.float32)
    Wo = np.asarray(inputs["Wo"], dtype=np.float32)
    bo = np.asarray(inputs["bo"], dtype=np.float32)

    idx = [np.nonzero(mask[b, 0, 0] != 0)[0] for b in range(B)]
    keff = [len(i) for i in idx]
    KP = max(P, ((max(keff) + P - 1) // P) * P)
    T = KP // P

    nc = _get_prog(KP)

    per_batch = {}
    for b in range(B):
        xqT = np.ascontiguousarray(query[b].T).astype(BF)
        xkT = np.zeros((DM, KP), dtype=BF)
        xkT[:, :keff[b]] = key[b][idx[b]].T.astype(BF)
        xvT = np.zeros((DM, KP), dtype=BF)
        xvT[:, :keff[b]] = value[b][idx[b]].T.astype(BF)
        vmf = np.zeros((KP,), dtype=np.float32)
        vmf[:keff[b]] = 1.0
        vm2 = np.ascontiguousarray(vmf.reshape(T, P).T)  # [128, T]
        per_batch[b] = (xqT, xkT, xvT, vm2)

    in_maps = []
    for core in range(NCORES):
        b, g = core // G, core % G
        xqT, xkT, xvT, vm2 = per_batch[b]
        sl = slice(g * DQ, (g + 1) * DQ)
        in_maps.append({
            "xqT": xqT,
            "xkT": xkT,
            "xvT": xvT,
            "wq": np.ascontiguousarray(Wq[:, sl]).astype(BF),
            "wk": np.ascontiguousarray(Wk[:, sl]).astype(BF),
            "wv": np.ascontiguousarray(Wv[:, sl]).astype(BF),
            "wo": np.ascontiguousarray(Wo[sl, :]).astype(BF),
            "bq": np.ascontiguousarray(bq[sl].reshape(DQ, 1)),
            "bk": np.ascontiguousarray(bk[sl].reshape(DQ, 1)),
            "bv": np.ascontiguousarray(bv[sl].reshape(1, DQ)),
            "vm": vm2,
        })

    res = run_bass_kernel_spmd(nc, in_maps, list(range(NCORES)), trace=trace)

    outp = np.zeros((B, S, DM), dtype=np.float32)
    for core in range(NCORES):
        outp[core // G] += res.results[core]["out"]
    outp += bo.reshape(1, 1, DM)
    return outp, res


def kernel(**inputs) -> np.ndarray:
    out, _ = _run(inputs, trace=False)
    return out


if __name__ == "__main__":
    nc = _build_nc(1152)
    print("build OK")



# revision 12
# speedup vs baseline: 1.0137x; 1.0137x over previous
"""Multi-head attention (B=2, S=2048, d_model=768, H=12) on 8 TRN2 NeuronCores.

Sharding: 2-way data parallel over batch x 4-way tensor parallel over heads
(3 heads / 192-wide d_model slice per core). Host compacts masked keys away
(gather of unmasked key/value rows), pads to a 128 multiple, and passes a 0/1
validity vector; softmax needs no mask handling on device (pad keys get V=0
and a 0 in the denominator ones-column).

v2 schedule (per core):
  - Weights host-preformatted to [128, 6*192] so every weight DMA is one
    large-line transfer (the v1 rearrange-DMA moved 384B packets at 10GB/s).
  - All PSUM->SBUF evictions moved off ACT onto DVE (tensor_scalar_add with
    the bias); ACT does exp only - it is the attention-phase bottleneck at
    (N+352)/1.2 ns per instruction.
  - Attention per 512-query chunk: loop A does heads 0/1 (scores paired into
    disjoint PE row groups, shared exp over [128,1024], PV into ctx0/ctx1),
    then h2 runs as a deferred loop B with its own score pairing via a
    partition-duplicated KT1/QT1 (rows 0:64 and 64:128) so two key tiles
    dual-issue. This keeps PSUM at 8 banks: scores 2x2, ctx 2x1, po 2x1.
  - Emission is software-pipelined: scores(t+1) enter the PE queue before
    PV(t) so the PE never head-of-line blocks on exp(t); O-projection of
    chunk c-1 and Q-projection of chunk c+1 are interleaved into chunk c's
    ACT-bound loop as PE filler work.
  - Output partials are written bf16 (halves the out DMA), summed on host.
"""

import math

import numpy as np

B = 2
S = 2048
DM = 768
H = 12
DH = 64
G = 4              # head-group (tensor-parallel) degree
HPG = H // G       # heads per core
DQ = HPG * DH      # 192 d_model slice per core
NCORES = 8
P = 128
NKT = DM // P      # 6 contraction tiles for projections
CW = 512           # query chunk width
NCH = S // CW      # 4 query chunks

_prog_cache = {}


def _chunks(total, step):
    out = []
    o = 0
    while o < total:
        w = min(step, total - o)
        out.append((o, w))
        o += w
    return out


def _build_nc(KP):
    import concourse.bass as bass
    import concourse.mybir as mybir
    import concourse.tile as tile
    from concourse import bacc

    F32 = mybir.dt.float32
    BF = mybir.dt.bfloat16
    AFT = mybir.ActivationFunctionType

    T = KP // P            # key tiles
    KCH = _chunks(KP, 512)

    nc = bacc.Bacc(None, target_bir_lowering=False)
    xqT = nc.declare_dram_parameter("xqT", [DM, S], BF, isOutput=False)
    xkT = nc.declare_dram_parameter("xkT", [DM, KP], BF, isOutput=False)
    xvT = nc.declare_dram_parameter("xvT", [DM, KP], BF, isOutput=False)
    # weights preformatted on host: w[p, kt*DQ+j] = W[kt*128+p, j]
    wq = nc.declare_dram_parameter("wq", [P, NKT * DQ], BF, isOutput=False)
    wk = nc.declare_dram_parameter("wk", [P, NKT * DQ], BF, isOutput=False)
    wv = nc.declare_dram_parameter("wv", [P, NKT * DQ], BF, isOutput=False)
    wo = nc.declare_dram_parameter("wo", [DQ, DM], BF, isOutput=False)
    bq = nc.declare_dram_parameter("bq", [DQ, 1], F32, isOutput=False)
    bk = nc.declare_dram_parameter("bk", [DQ, 1], F32, isOutput=False)
    bv = nc.declare_dram_parameter("bv", [1, DQ], F32, isOutput=False)
    vm = nc.declare_dram_parameter("vm", [P, T], F32, isOutput=False)
    out = nc.declare_dram_parameter("out", [S, DM], BF, isOutput=True)

    scale = 1.0 / math.sqrt(DH)

    with tile.TileContext(nc) as tc:
        with (
            tc.tile_pool(name="persist", bufs=1) as persist,
            tc.tile_pool(name="acts", bufs=6) as acts,
            tc.tile_pool(name="es", bufs=6) as espool,
            tc.tile_pool(name="norm", bufs=2) as norm,
            tc.tile_pool(name="osb", bufs=3) as osb,
            tc.tile_pool(name="sp_ps", bufs=2, space="PSUM") as sp_ps,
            tc.tile_pool(name="ctx_ps", bufs=2, space="PSUM") as ctx_ps,
            tc.tile_pool(name="po_ps", bufs=2, space="PSUM") as po_ps,
        ):
            # ---- persistent tiles ----
            WQ = persist.tile([P, NKT * DQ], BF, tag="WQ")
            WK = persist.tile([P, NKT * DQ], BF, tag="WK")
            WV = persist.tile([P, NKT * DQ], BF, tag="WV")
            WO0 = persist.tile([P, DM], BF, tag="WO0")   # wo rows 0:128 (h0,h1)
            WO2 = persist.tile([DH, DM], BF, tag="WO2")  # wo rows 128:192 (h2)
            BQ0 = persist.tile([P, 1], F32, tag="BQ0")
            BQ1 = persist.tile([DH, 1], F32, tag="BQ1")
            BK0 = persist.tile([P, 1], F32, tag="BK0")
            BK1 = persist.tile([DH, 1], F32, tag="BK1")
            BV = persist.tile([P, DQ], F32, tag="BV")
            VM = persist.tile([P, T], F32, tag="VM")
            QT0 = persist.tile([P, S], BF, tag="QT0")    # heads 0,1
            QT1 = persist.tile([DH, S], BF, tag="QT1")   # head 2
            KT0 = persist.tile([P, KP], BF, tag="KT0")
            KT1 = persist.tile([DH, KP], BF, tag="KT1")
            # V blocks per (t, head): [V_h(64) | valid(1) | zero(63)]
            VP = persist.tile([P, T, HPG * P], BF, tag="VP")
            CTX01 = persist.tile([P, S], BF, tag="CTX01")  # h0 rows 0:64, h1 64:128
            CTX2 = persist.tile([DH, S], BF, tag="CTX2")

            # ---- DMA issue, priority order ----
            nc.sync.dma_start(out=WK, in_=wk[:, :])
            XK = []
            for kt in range(NKT):
                xt = acts.tile([P, KP], BF, tag="xk", name=f"xk{kt}")
                nc.sync.dma_start(out=xt, in_=xkT[kt * P:(kt + 1) * P, :])
                XK.append(xt)
            nc.sync.dma_start(out=WQ, in_=wq[:, :])
            XQ = []
            for kt in range(NKT):
                xt = acts.tile([P, S], BF, tag="xq", name=f"xq{kt}")
                nc.sync.dma_start(out=xt[:, 0:CW], in_=xqT[kt * P:(kt + 1) * P, 0:CW])
                XQ.append(xt)
            nc.sync.dma_start(out=BQ0, in_=bq[0:P, :])
            nc.sync.dma_start(out=BQ1, in_=bq[P:DQ, :])
            nc.sync.dma_start(out=BK0, in_=bk[0:P, :])
            nc.sync.dma_start(out=BK1, in_=bk[P:DQ, :])
            nc.sync.dma_start(out=BV, in_=bv[:, :].to_broadcast([P, DQ]))
            nc.sync.dma_start(out=VM, in_=vm[:, :])
            nc.sync.dma_start(out=WV, in_=wv[:, :])
            XV = []
            for kt in range(NKT):
                xt = acts.tile([P, KP], BF, tag="xv", name=f"xv{kt}")
                nc.sync.dma_start(out=xt, in_=xvT[kt * P:(kt + 1) * P, :])
                XV.append(xt)
            for c0 in range(CW, S, CW):
                for kt in range(NKT):
                    nc.sync.dma_start(
                        out=XQ[kt][:, c0:c0 + CW],
                        in_=xqT[kt * P:(kt + 1) * P, c0:c0 + CW],
                    )
            nc.sync.dma_start(out=WO0, in_=wo[0:P, :])
            nc.sync.dma_start(out=WO2, in_=wo[P:DQ, :])
            nc.vector.memset(VP, 0.0)

            # ---- phase A: K projection (DVE evicts; ACT stays free for exp) ----
            for ci, (c0, cwk) in enumerate(KCH):
                kp0 = ctx_ps.tile([P, CW], F32, tag="ctx", name=f"kp0_{ci}")
                for kt in range(NKT):
                    nc.tensor.matmul(
                        kp0[:, 0:cwk],
                        lhsT=WK[:, kt * DQ:kt * DQ + P],
                        rhs=XK[kt][:, c0:c0 + cwk],
                        start=(kt == 0), stop=(kt == NKT - 1),
                    )
                nc.vector.tensor_scalar_add(
                    KT0[:, c0:c0 + cwk], kp0[:, 0:cwk], BK0
                )
                kp1 = ctx_ps.tile([P, CW], F32, tag="ctx", name=f"kp1_{ci}")
                for kt in range(NKT):
                    nc.tensor.matmul(
                        kp1[0:DH, 0:cwk],
                        lhsT=WK[:, kt * DQ + P:kt * DQ + DQ],
                        rhs=XK[kt][:, c0:c0 + cwk],
                        start=(kt == 0), stop=(kt == NKT - 1),
                    )
                nc.vector.tensor_scalar_add(
                    KT1[:, c0:c0 + cwk], kp1[0:DH, 0:cwk], BK1
                )

            def qproj(c0, which):
                """which 0 -> heads 0/1 slice, 1 -> head 2 slice (+dup)."""
                qp = po_ps.tile([P, CW], F32, tag="po", name=f"qp{which}_{c0}")
                if which == 0:
                    for kt in range(NKT):
                        nc.tensor.matmul(
                            qp[:, :],
                            lhsT=WQ[:, kt * DQ:kt * DQ + P],
                            rhs=XQ[kt][:, c0:c0 + CW],
                            start=(kt == 0), stop=(kt == NKT - 1),
                        )
                    nc.vector.tensor_scalar_add(QT0[:, c0:c0 + CW], qp[:, :], BQ0)
                else:
                    for kt in range(NKT):
                        nc.tensor.matmul(
                            qp[0:DH, :],
                            lhsT=WQ[:, kt * DQ + P:kt * DQ + DQ],
                            rhs=XQ[kt][:, c0:c0 + CW],
                            start=(kt == 0), stop=(kt == NKT - 1),
                        )
                    nc.vector.tensor_scalar_add(
                        QT1[:, c0:c0 + CW], qp[0:DH, :], BQ1
                    )

            def vproj(t):
                vp = po_ps.tile([P, CW], F32, tag="po", name=f"vp{t}")
                for kt in range(NKT):
                    nc.tensor.matmul(
                        vp[:, 0:DQ],
                        lhsT=XV[kt][:, t * P:(t + 1) * P],
                        rhs=WV[:, kt * DQ:(kt + 1) * DQ],
                        start=(kt == 0), stop=(kt == NKT - 1),
                    )
                vv = VP[:, t, :].rearrange("p (h c) -> p h c", c=P)
                nc.vector.tensor_add(
                    vv[:, :, 0:DH],
                    vp[:, 0:DQ].rearrange("p (h d) -> p h d", d=DH),
                    BV[:, :].rearrange("p (h d) -> p h d", d=DH),
                )
                nc.vector.tensor_scalar_mul(
                    vv[:, :, 0:DH], vv[:, :, 0:DH], VM[:, t:t + 1]
                )
                nc.vector.tensor_copy(
                    vv[:, :, DH:DH + 1],
                    VM[:, t:t + 1].to_broadcast([P, HPG, 1]),
                )

            def oproj(m):
                lhA = CTX01[:, m * P:(m + 1) * P]
                lhB = CTX2[:, m * P:(m + 1) * P]
                po_a = po_ps.tile([P, CW], F32, tag="po", name=f"poa{m}")
                nc.tensor.matmul(po_a[:, :], lhsT=lhA, rhs=WO0[:, 0:CW],
                                 start=True, stop=False)
                nc.tensor.matmul(po_a[:, :], lhsT=lhB, rhs=WO2[:, 0:CW],
                                 start=False, stop=True)
                po_b = po_ps.tile([P, CW], F32, tag="po", name=f"pob{m}")
                nc.tensor.matmul(po_b[:, 0:DM - CW], lhsT=lhA, rhs=WO0[:, CW:DM],
                                 start=True, stop=False)
                nc.tensor.matmul(po_b[:, 0:DM - CW], lhsT=lhB, rhs=WO2[:, CW:DM],
                                 start=False, stop=True)
                psb = osb.tile([P, DM], BF, tag="posb", name=f"psb{m}")
                nc.vector.tensor_copy(psb[:, 0:CW], po_a[:, :])
                nc.vector.tensor_copy(psb[:, CW:DM], po_b[:, 0:DM - CW])
                nc.sync.dma_start(out=out[m * P:(m + 1) * P, :], in_=psb)

            # ---- phase A tail: Q projection for chunk 0 ----
            qproj(0, 0)
            qproj(0, 1)

            # ---- attention chunks ----
            def scores01(ci, t):
                sp = sp_ps.tile([P, 2 * CW], F32, tag="sp", name=f"sp{ci}_{t}")
                c0 = ci * CW
                nc.tensor.matmul(
                    sp[:, 0:CW],
                    lhsT=KT0[0:DH, t * P:(t + 1) * P],
                    rhs=QT0[0:DH, c0:c0 + CW],
                    start=True, stop=True,
                )
                nc.tensor.matmul(
                    sp[:, CW:2 * CW],
                    lhsT=KT0[DH:P, t * P:(t + 1) * P],
                    rhs=QT0[DH:P, c0:c0 + CW],
                    start=True, stop=True,
                )
                return sp

            for ci in range(NCH):
                c0 = ci * CW
                ctx0 = ctx_ps.tile([P, CW], F32, tag="ctx", name=f"c0_{ci}")
                ctx1 = ctx_ps.tile([P, CW], F32, tag="ctx", name=f"c1_{ci}")

                def pv01(t, es):
                    nc.tensor.matmul(
                        ctx0[:, :],
                        lhsT=VP[:, t, 0:P],
                        rhs=es[:, 0:CW],
                        start=(t == 0), stop=(t == T - 1),
                    )
                    nc.tensor.matmul(
                        ctx1[:, :],
                        lhsT=VP[:, t, P:2 * P],
                        rhs=es[:, CW:2 * CW],
                        start=(t == 0), stop=(t == T - 1),
                    )

                def do_exp(sp, t):
                    es = espool.tile([P, 2 * CW], BF, tag="es", name=f"es{ci}_{t}")
                    nc.scalar.activation(es, sp, AFT.Exp, bias=0.0, scale=scale)
                    return es

                extras = {}
                if ci == 0:
                    # Deep pipeline: XV lands well after XQ, so scores/exp run
                    # ahead while vproj(t)/pv(t-1) trail one step behind.
                    sps = {0: scores01(ci, 0), 1: scores01(ci, 1)}
                    ess = {0: do_exp(sps.pop(0), 0)}
                    for t in range(T):
                        if t + 2 < T:
                            sps[t + 2] = scores01(ci, t + 2)
                        if t + 1 < T:
                            ess[t + 1] = do_exp(sps.pop(t + 1), t + 1)
                        vproj(t)
                        if t >= 1:
                            pv01(t - 1, ess.pop(t - 1))
                    pv01(T - 1, ess.pop(T - 1))
                else:
                    mlist = [(ci - 1) * 4 + i for i in range(4)]
                    for sl, m in zip([2, 4, 6, 8], mlist):
                        extras[sl] = (lambda mm: (lambda: oproj(mm)))(m)
                    if ci < NCH - 1:
                        nc0 = (ci + 1) * CW
                        extras[3] = (lambda cc: (lambda: qproj(cc, 0)))(nc0)
                        extras[5] = (lambda cc: (lambda: qproj(cc, 1)))(nc0)
                    sp_cur = scores01(ci, 0)
                    for t in range(T):
                        sp_next = scores01(ci, t + 1) if t + 1 < T else None
                        es = do_exp(sp_cur, t)
                        if t + 2 in extras:
                            extras.pop(t + 2)()
                        pv01(t, es)
                        sp_cur = sp_next

                # free ctx0/ctx1: evict numerators + denominators (SBUF), then
                # reciprocals from SBUF. All tensor-tensor inputs share base
                # partition 0 (HW requires equal SB input bases); only outputs
                # may be partition-offset (baseline-proven).
                cs0 = norm.tile([DH, CW], F32, tag="cs0", name=f"cs0_{ci}")
                cs1 = norm.tile([DH, CW], F32, tag="cs1", name=f"cs1_{ci}")
                nc.vector.tensor_copy(cs0, ctx0[0:DH, :])
                nc.vector.tensor_copy(cs1, ctx1[0:DH, :])
                dn = norm.tile([1, 3 * CW], F32, tag="dn", name=f"dn{ci}")
                nc.vector.tensor_copy(dn[:, 0:CW], ctx0[DH:DH + 1, :])
                nc.vector.tensor_copy(dn[:, CW:2 * CW], ctx1[DH:DH + 1, :])
                rc = norm.tile([1, 3 * CW], F32, tag="rc", name=f"rc{ci}")
                nc.vector.reciprocal_approx_fast(rc[:, 0:2 * CW], dn[:, 0:2 * CW])

                # loop B: head 2, deferred; paired scores via duplicated KT1/QT1
                ctx2 = ctx_ps.tile([P, CW], F32, tag="ctx", name=f"c2_{ci}")
                for tg0 in range(0, T, 2):
                    tl = [tg0, tg0 + 1] if tg0 + 1 < T else [tg0]
                    ln = len(tl)
                    sp2 = sp_ps.tile([P, 2 * CW], F32, tag="sp", name=f"sp2_{ci}_{tg0}")
                    for i, t in enumerate(tl):
                        nc.tensor.matmul(
                            sp2[:, i * CW:(i + 1) * CW],
                            lhsT=KT1[:, t * P:(t + 1) * P],
                            rhs=QT1[:, c0:c0 + CW],
                            start=True, stop=True,
                        )
                    es2 = espool.tile([P, 2 * CW], BF, tag="es", name=f"es2_{ci}_{tg0}")
                    nc.scalar.activation(
                        es2[:, 0:ln * CW], sp2[:, 0:ln * CW],
                        AFT.Exp, bias=0.0, scale=scale,
                    )
                    if tg0 + 2 >= T and extras:
                        for k in sorted(extras):
                            extras.pop(k)()
                    for i, t in enumerate(tl):
                        nc.tensor.matmul(
                            ctx2[:, :],
                            lhsT=VP[:, t, 2 * P:3 * P],
                            rhs=es2[:, i * CW:(i + 1) * CW],
                            start=(t == 0), stop=(t == T - 1),
                        )

                cs2 = norm.tile([DH, CW], F32, tag="cs2", name=f"cs2_{ci}")
                nc.vector.tensor_copy(cs2, ctx2[0:DH, :])
                nc.vector.tensor_copy(dn[:, 2 * CW:3 * CW], ctx2[DH:DH + 1, :])
                nc.vector.reciprocal_approx_fast(rc[:, 2 * CW:3 * CW], dn[:, 2 * CW:3 * CW])
                bc0 = norm.tile([DH, CW], F32, tag="bc0", name=f"bc0_{ci}")
                bc1 = norm.tile([DH, CW], F32, tag="bc1", name=f"bc1_{ci}")
                bc2 = norm.tile([DH, CW], F32, tag="bc2", name=f"bc2_{ci}")
                nc.gpsimd.partition_broadcast(bc0, rc[:, 0:CW])
                nc.vector.tensor_mul(CTX01[0:DH, c0:c0 + CW], cs0, bc0)
                nc.gpsimd.partition_broadcast(bc1, rc[:, CW:2 * CW])
                nc.vector.tensor_mul(CTX01[DH:P, c0:c0 + CW], cs1, bc1)
                nc.gpsimd.partition_broadcast(bc2, rc[:, 2 * CW:3 * CW])
                nc.vector.tensor_mul(CTX2[:, c0:c0 + CW], cs2, bc2)
                # Q projection for the next chunk runs on PE while DVE/gpsimd
                # finish this chunk's normalization.
                if ci == 0 and NCH > 1:
                    qproj(CW, 0)
                    qproj(CW, 1)

            # tail: O-projection of the last chunk
            for m in range((NCH - 1) * 4, NCH * 4):
                oproj(m)
    nc.compile()
    return nc


def _get_prog(KP):
    if KP not in _prog_cache:
        _prog_cache[KP] = _build_nc(KP)
    return _prog_cache[KP]


def _fmt_w(w):
    # [768, 192] -> [128, 6*192]: row kt*128+p, col j  ->  [p, kt*192+j]
    import ml_dtypes
    return np.ascontiguousarray(
        w.reshape(NKT, P, DQ).transpose(1, 0, 2).reshape(P, NKT * DQ)
    ).astype(ml_dtypes.bfloat16)


def _run(inputs, trace=False):
    import ml_dtypes
    from concourse.bass_utils import run_bass_kernel_spmd

    BF = ml_dtypes.bfloat16

    query = np.asarray(inputs["query"], dtype=np.float32)
    key = np.asarray(inputs["key"], dtype=np.float32)
    value = np.asarray(inputs["value"], dtype=np.float32)
    mask = np.asarray(inputs["mask"])
    Wq = np.asarray(inputs["Wq"], dtype=np.float32)
    bq = np.asarray(inputs["bq"], dtype=np.float32)
    Wk = np.asarray(inputs["Wk"], dtype=np.float32)
    bk = np.asarray(inputs["bk"], dtype=np.float32)
    Wv = np.asarray(inputs["Wv"], dtype=np.float32)
    bv = np.asarray(inputs["bv"], dtype=np.float32)
    Wo = np.asarray(inputs["Wo"], dtype=np.float32)
    bo = np.asarray(inputs["bo"], dtype=np.float32)

    idx = [np.nonzero(mask[b, 0, 0] != 0)[0] for b in range(B)]
    keff = [len(i) for i in idx]
    KP = max(P, ((max(keff) + P - 1) // P) * P)
    T = KP // P

    nc = _get_prog(KP)

    per_batch = {}
    for b in range(B):
        xqT = np.ascontiguousarray(query[b].T).astype(BF)
        xkT = np.zeros((DM, KP), dtype=BF)
        xkT[:, :keff[b]] = key[b][idx[b]].T.astype(BF)
        xvT = np.zeros((DM, KP), dtype=BF)
        xvT[:, :keff[b]] = value[b][idx[b]].T.astype(BF)
        vmf = np.zeros((KP,), dtype=np.float32)
        vmf[:keff[b]] = 1.0
        vm2 = np.ascontiguousarray(vmf.reshape(T, P).T)  # [128, T]
        per_batch[b] = (xqT, xkT, xvT, vm2)

    in_maps = []
    for core in range(NCORES):
        b, g = core // G, core % G
        xqT, xkT, xvT, vm2 = per_batch[b]
        sl = slice(g * DQ, (g + 1) * DQ)
        in_maps.append({
            "xqT": xqT,
            "xkT": xkT,
            "xvT": xvT,
            "wq": _fmt_w(Wq[:, sl]),
            "wk": _fmt_w(Wk[:, sl]),
            "wv": _fmt_w(Wv[:, sl]),
            "wo": np.ascontiguousarray(Wo[sl, :]).astype(BF),
            "bq": np.ascontiguousarray(bq[sl].reshape(DQ, 1)),
            "bk": np.ascontiguousarray(bk[sl].reshape(DQ, 1)),
            "bv": np.ascontiguousarray(bv[sl].reshape(1, DQ)),
            "vm": vm2,
        })

    res = run_bass_kernel_spmd(nc, in_maps, list(range(NCORES)), trace=trace)

    outp = np.zeros((B, S, DM), dtype=np.float32)
    for core in range(NCORES):
        outp[core // G] += res.results[core]["out"].astype(np.float32)
    outp += bo.reshape(1, 1, DM)
    return outp, res


def kernel(**inputs) -> np.ndarray:
    out, _ = _run(inputs, trace=False)
    return out


if __name__ == "__main__":
    nc = _build_nc(1152)
    print("build OK")


# revision 17
# speedup vs baseline: 1.1775x; 1.1616x over previous
"""Multi-head attention (B=2, S=2048, d_model=768, H=12) on 8 TRN2 NeuronCores.

Sharding: 2-way data parallel over batch x 4-way tensor parallel over heads
(3 heads / 192-wide d_model slice per core). Host compacts masked keys away
(gather of unmasked key/value rows), pads to a 128 multiple, and passes a 0/1
validity vector; softmax needs no mask handling on device (pad keys get V=0
and a 0 in the denominator ones-column).

v2 schedule (per core):
  - Weights host-preformatted to [128, 6*192] so every weight DMA is one
    large-line transfer (the v1 rearrange-DMA moved 384B packets at 10GB/s).
  - All PSUM->SBUF evictions moved off ACT onto DVE (tensor_scalar_add with
    the bias); ACT does exp only - it is the attention-phase bottleneck at
    (N+352)/1.2 ns per instruction.
  - Attention per 512-query chunk: loop A does heads 0/1 (scores paired into
    disjoint PE row groups, shared exp over [128,1024], PV into ctx0/ctx1),
    then h2 runs as a deferred loop B with its own score pairing via a
    partition-duplicated KT1/QT1 (rows 0:64 and 64:128) so two key tiles
    dual-issue. This keeps PSUM at 8 banks: scores 2x2, ctx 2x1, po 2x1.
  - Emission is software-pipelined: scores(t+1) enter the PE queue before
    PV(t) so the PE never head-of-line blocks on exp(t); O-projection of
    chunk c-1 and Q-projection of chunk c+1 are interleaved into chunk c's
    ACT-bound loop as PE filler work.
  - Output partials are written bf16 (halves the out DMA), summed on host.
"""

import math

import numpy as np

B = 2
S = 2048
DM = 768
H = 12
DH = 64
G = 4              # head-group (tensor-parallel) degree
HPG = H // G       # heads per core
DQ = HPG * DH      # 192 d_model slice per core
NCORES = 8
P = 128
NKT = DM // P      # 6 contraction tiles for projections
CW = 512           # query chunk width
NCH = S // CW      # 4 query chunks

_prog_cache = {}


def _chunks(total, step):
    out = []
    o = 0
    while o < total:
        w = min(step, total - o)
        out.append((o, w))
        o += w
    return out


def _build_nc(KP):
    import concourse.bass as bass
    import concourse.mybir as mybir
    import concourse.tile as tile
    from concourse import bacc

    F32 = mybir.dt.float32
    BF = mybir.dt.bfloat16
    AFT = mybir.ActivationFunctionType

    T = KP // P            # key tiles
    KCH = _chunks(KP, 512)

    nc = bacc.Bacc(None, target_bir_lowering=False)
    # all inputs host-preformatted into few large contiguous transfers
    # (each DMA_DIRECT2D costs ~0.6us of serialized sync-engine issue)
    xqf = nc.declare_dram_parameter("xqf", [P, NCH * NKT * CW], BF, isOutput=False)
    xkf = nc.declare_dram_parameter("xkf", [P, NKT * KP], BF, isOutput=False)
    xvf = nc.declare_dram_parameter("xvf", [P, NKT * KP], BF, isOutput=False)
    wq = nc.declare_dram_parameter("wq", [P, NKT * DQ], BF, isOutput=False)
    wk = nc.declare_dram_parameter("wk", [P, NKT * DQ], BF, isOutput=False)
    wv = nc.declare_dram_parameter("wv", [P, NKT * DQ], BF, isOutput=False)
    wof = nc.declare_dram_parameter("wof", [P, 2 * DM], BF, isOutput=False)
    NCONS = 4 + T + DQ
    cons = nc.declare_dram_parameter("cons", [P, NCONS], F32, isOutput=False)
    out = nc.declare_dram_parameter("out", [S, DM], BF, isOutput=True)

    scale = 1.0 / math.sqrt(DH)

    with tile.TileContext(nc) as tc:
        with (
            tc.tile_pool(name="persist", bufs=1) as persist,
            tc.tile_pool(name="es", bufs=6) as espool,
            tc.tile_pool(name="norm", bufs=2) as norm,
            tc.tile_pool(name="osb", bufs=3) as osb,
            tc.tile_pool(name="sp_ps", bufs=2, space="PSUM") as sp_ps,
            tc.tile_pool(name="ctx_ps", bufs=2, space="PSUM") as ctx_ps,
            tc.tile_pool(name="po_ps", bufs=2, space="PSUM") as po_ps,
        ):
            # ---- persistent tiles ----
            WQ = persist.tile([P, NKT * DQ], BF, tag="WQ")
            WK = persist.tile([P, NKT * DQ], BF, tag="WK")
            WV = persist.tile([P, NKT * DQ], BF, tag="WV")
            WOF = persist.tile([P, 2 * DM], BF, tag="WOF")
            WO0 = WOF[:, 0:DM]           # wo rows 0:128 (h0,h1)
            WO2 = WOF[0:DH, DM:2 * DM]   # wo rows 128:192 (h2)
            CONS = persist.tile([P, NCONS], F32, tag="CONS")
            BQ0 = CONS[:, 0:1]
            BQ1 = CONS[0:DH, 1:2]
            BK0 = CONS[:, 2:3]
            BK1 = CONS[0:DH, 3:4]
            VM = CONS[:, 4:4 + T]
            BV = CONS[:, 4 + T:4 + T + DQ]
            QT0 = persist.tile([P, S], BF, tag="QT0")    # heads 0,1
            QT1 = persist.tile([DH, S], BF, tag="QT1")   # head 2
            KT0 = persist.tile([P, KP], BF, tag="KT0")
            KT1 = persist.tile([DH, KP], BF, tag="KT1")
            # V blocks per (t, head): [V_h(64) | valid(1) | zero(63)]
            VP = persist.tile([P, T, HPG * P], BF, tag="VP")
            CTX01 = persist.tile([P, S], BF, tag="CTX01")  # h0 rows 0:64, h1 64:128
            CTX2 = persist.tile([DH, S], BF, tag="CTX2")

            XKb = persist.tile([P, NKT, KP], BF, tag="XKb")
            XVb = persist.tile([P, NKT, KP], BF, tag="XVb")
            XQb = persist.tile([P, NCH, NKT, CW], BF, tag="XQb")

            # ---- DMA issue, priority order ----
            nc.sync.dma_start(out=WK, in_=wk[:, :])
            nc.sync.dma_start(
                out=XKb, in_=xkf[:, :].rearrange("p (kt j) -> p kt j", j=KP)
            )
            nc.sync.dma_start(out=WQ, in_=wq[:, :])
            nc.sync.dma_start(
                out=XQb[:, 0, :, :],
                in_=xqf[:, 0:NKT * CW].rearrange("p (kt j) -> p kt j", j=CW),
            )
            nc.sync.dma_start(out=CONS, in_=cons[:, :])
            nc.sync.dma_start(out=WV, in_=wv[:, :])
            nc.sync.dma_start(
                out=XVb, in_=xvf[:, :].rearrange("p (kt j) -> p kt j", j=KP)
            )
            for c in range(1, NCH):
                nc.sync.dma_start(
                    out=XQb[:, c, :, :],
                    in_=xqf[:, c * NKT * CW:(c + 1) * NKT * CW].rearrange(
                        "p (kt j) -> p kt j", j=CW
                    ),
                )
            nc.sync.dma_start(out=WOF, in_=wof[:, :])
            nc.vector.memset(VP, 0.0)
            XK = [XKb[:, kt, :] for kt in range(NKT)]
            XV = [XVb[:, kt, :] for kt in range(NKT)]

            # ---- phase A: K projection (DVE evicts; ACT stays free for exp) ----
            for ci, (c0, cwk) in enumerate(KCH):
                kp0 = ctx_ps.tile([P, CW], F32, tag="ctx", name=f"kp0_{ci}")
                for kt in range(NKT):
                    nc.tensor.matmul(
                        kp0[:, 0:cwk],
                        lhsT=WK[:, kt * DQ:kt * DQ + P],
                        rhs=XK[kt][:, c0:c0 + cwk],
                        start=(kt == 0), stop=(kt == NKT - 1),
                    )
                nc.vector.tensor_scalar_add(
                    KT0[:, c0:c0 + cwk], kp0[:, 0:cwk], BK0
                )
                kp1 = ctx_ps.tile([P, CW], F32, tag="ctx", name=f"kp1_{ci}")
                for kt in range(NKT):
                    nc.tensor.matmul(
                        kp1[0:DH, 0:cwk],
                        lhsT=WK[:, kt * DQ + P:kt * DQ + DQ],
                        rhs=XK[kt][:, c0:c0 + cwk],
                        start=(kt == 0), stop=(kt == NKT - 1),
                    )
                nc.vector.tensor_scalar_add(
                    KT1[:, c0:c0 + cwk], kp1[0:DH, 0:cwk], BK1
                )

            def qproj(c0, which):
                """which 0 -> heads 0/1 slice, 1 -> head 2 slice (+dup)."""
                qp = po_ps.tile([P, CW], F32, tag="po", name=f"qp{which}_{c0}")
                if which == 0:
                    for kt in range(NKT):
                        nc.tensor.matmul(
                            qp[:, :],
                            lhsT=WQ[:, kt * DQ:kt * DQ + P],
                            rhs=XQb[:, c0 // CW, kt, :],
                            start=(kt == 0), stop=(kt == NKT - 1),
                        )
                    nc.vector.tensor_scalar_add(QT0[:, c0:c0 + CW], qp[:, :], BQ0)
                else:
                    for kt in range(NKT):
                        nc.tensor.matmul(
                            qp[0:DH, :],
                            lhsT=WQ[:, kt * DQ + P:kt * DQ + DQ],
                            rhs=XQb[:, c0 // CW, kt, :],
                            start=(kt == 0), stop=(kt == NKT - 1),
                        )
                    nc.vector.tensor_scalar_add(
                        QT1[:, c0:c0 + CW], qp[0:DH, :], BQ1
                    )

            def vproj(t):
                vp = po_ps.tile([P, CW], F32, tag="po", name=f"vp{t}")
                for kt in range(NKT):
                    nc.tensor.matmul(
                        vp[:, 0:DQ],
                        lhsT=XV[kt][:, t * P:(t + 1) * P],
                        rhs=WV[:, kt * DQ:(kt + 1) * DQ],
                        start=(kt == 0), stop=(kt == NKT - 1),
                    )
                vv = VP[:, t, :].rearrange("p (h c) -> p h c", c=P)
                nc.vector.tensor_add(
                    vv[:, :, 0:DH],
                    vp[:, 0:DQ].rearrange("p (h d) -> p h d", d=DH),
                    BV[:, :].rearrange("p (h d) -> p h d", d=DH),
                )
                nc.vector.tensor_scalar_mul(
                    vv[:, :, 0:DH], vv[:, :, 0:DH], VM[:, t:t + 1]
                )
                nc.vector.tensor_copy(
                    vv[:, :, DH:DH + 1],
                    VM[:, t:t + 1].to_broadcast([P, HPG, 1]),
                )

            def oproj(m):
                lhA = CTX01[:, m * P:(m + 1) * P]
                lhB = CTX2[:, m * P:(m + 1) * P]
                po_a = po_ps.tile([P, CW], F32, tag="po", name=f"poa{m}")
                nc.tensor.matmul(po_a[:, :], lhsT=lhA, rhs=WO0[:, 0:CW],
                                 start=True, stop=False)
                nc.tensor.matmul(po_a[:, :], lhsT=lhB, rhs=WO2[:, 0:CW],
                                 start=False, stop=True)
                po_b = po_ps.tile([P, CW], F32, tag="po", name=f"pob{m}")
                nc.tensor.matmul(po_b[:, 0:DM - CW], lhsT=lhA, rhs=WO0[:, CW:DM],
                                 start=True, stop=False)
                nc.tensor.matmul(po_b[:, 0:DM - CW], lhsT=lhB, rhs=WO2[:, CW:DM],
                                 start=False, stop=True)
                psb = osb.tile([P, DM], BF, tag="posb", name=f"psb{m}")
                nc.vector.tensor_copy(psb[:, 0:CW], po_a[:, :])
                nc.vector.tensor_copy(psb[:, CW:DM], po_b[:, 0:DM - CW])
                nc.sync.dma_start(out=out[m * P:(m + 1) * P, :], in_=psb)

            # ---- phase A tail: Q projection for chunk 0 ----
            qproj(0, 0)
            qproj(0, 1)

            # ---- attention chunks ----
            def scores01(ci, t):
                sp = sp_ps.tile([P, 2 * CW], F32, tag="sp", name=f"sp{ci}_{t}")
                c0 = ci * CW
                nc.tensor.matmul(
                    sp[:, 0:CW],
                    lhsT=KT0[0:DH, t * P:(t + 1) * P],
                    rhs=QT0[0:DH, c0:c0 + CW],
                    start=True, stop=True,
                )
                nc.tensor.matmul(
                    sp[:, CW:2 * CW],
                    lhsT=KT0[DH:P, t * P:(t + 1) * P],
                    rhs=QT0[DH:P, c0:c0 + CW],
                    start=True, stop=True,
                )
                return sp

            for ci in range(NCH):
                c0 = ci * CW
                ctx0 = ctx_ps.tile([P, CW], F32, tag="ctx", name=f"c0_{ci}")
                ctx1 = ctx_ps.tile([P, CW], F32, tag="ctx", name=f"c1_{ci}")

                def pv01(t, es):
                    nc.tensor.matmul(
                        ctx0[:, :],
                        lhsT=VP[:, t, 0:P],
                        rhs=es[:, 0:CW],
                        start=(t == 0), stop=(t == T - 1),
                    )
                    nc.tensor.matmul(
                        ctx1[:, :],
                        lhsT=VP[:, t, P:2 * P],
                        rhs=es[:, CW:2 * CW],
                        start=(t == 0), stop=(t == T - 1),
                    )

                def do_exp(sp, t):
                    es = espool.tile([P, 2 * CW], BF, tag="es", name=f"es{ci}_{t}")
                    nc.scalar.activation(es, sp, AFT.Exp, bias=0.0, scale=scale)
                    return es

                extras = {}
                if ci == 0:
                    # Deep pipeline: XV lands well after XQ, so scores/exp run
                    # ahead while vproj(t)/pv(t-1) trail one step behind.
                    sps = {0: scores01(ci, 0), 1: scores01(ci, 1)}
                    ess = {0: do_exp(sps.pop(0), 0)}
                    for t in range(T):
                        if t + 2 < T:
                            sps[t + 2] = scores01(ci, t + 2)
                        if t + 1 < T:
                            ess[t + 1] = do_exp(sps.pop(t + 1), t + 1)
                        vproj(t)
                        if t >= 1:
                            pv01(t - 1, ess.pop(t - 1))
                    pv01(T - 1, ess.pop(T - 1))
                else:
                    mlist = [(ci - 1) * 4 + i for i in range(4)]
                    for sl, m in zip([2, 4, 6, 8], mlist):
                        extras[sl] = (lambda mm: (lambda: oproj(mm)))(m)
                    if ci < NCH - 1:
                        nc0 = (ci + 1) * CW
                        extras[3] = (lambda cc: (lambda: qproj(cc, 0)))(nc0)
                        extras[5] = (lambda cc: (lambda: qproj(cc, 1)))(nc0)
                    sp_cur = scores01(ci, 0)
                    for t in range(T):
                        sp_next = scores01(ci, t + 1) if t + 1 < T else None
                        es = do_exp(sp_cur, t)
                        if t + 2 in extras:
                            extras.pop(t + 2)()
                        pv01(t, es)
                        sp_cur = sp_next

                # free ctx0/ctx1: evict numerators + denominators (SBUF), then
                # reciprocals from SBUF. All tensor-tensor inputs share base
                # partition 0 (HW requires equal SB input bases); only outputs
                # may be partition-offset (baseline-proven).
                cs0 = norm.tile([DH, CW], F32, tag="cs0", name=f"cs0_{ci}")
                cs1 = norm.tile([DH, CW], F32, tag="cs1", name=f"cs1_{ci}")
                nc.vector.tensor_copy(cs0, ctx0[0:DH, :])
                nc.vector.tensor_copy(cs1, ctx1[0:DH, :])
                dn = norm.tile([1, 3 * CW], F32, tag="dn", name=f"dn{ci}")
                nc.vector.tensor_copy(dn[:, 0:CW], ctx0[DH:DH + 1, :])
                nc.vector.tensor_copy(dn[:, CW:2 * CW], ctx1[DH:DH + 1, :])
                rc = norm.tile([1, 3 * CW], F32, tag="rc", name=f"rc{ci}")
                nc.vector.reciprocal_approx_fast(rc[:, 0:2 * CW], dn[:, 0:2 * CW])

                # loop B: head 2, deferred; paired scores via duplicated KT1/QT1
                ctx2 = po_ps.tile([P, CW], F32, tag="po", name=f"c2_{ci}")
                for tg0 in range(0, T, 2):
                    tl = [tg0, tg0 + 1] if tg0 + 1 < T else [tg0]
                    ln = len(tl)
                    sp2 = sp_ps.tile([P, 2 * CW], F32, tag="sp", name=f"sp2_{ci}_{tg0}")
                    for i, t in enumerate(tl):
                        nc.tensor.matmul(
                            sp2[:, i * CW:(i + 1) * CW],
                            lhsT=KT1[:, t * P:(t + 1) * P],
                            rhs=QT1[:, c0:c0 + CW],
                            start=True, stop=True,
                        )
                    es2 = espool.tile([P, 2 * CW], BF, tag="es", name=f"es2_{ci}_{tg0}")
                    nc.scalar.activation(
                        es2[:, 0:ln * CW], sp2[:, 0:ln * CW],
                        AFT.Exp, bias=0.0, scale=scale,
                    )
                    if tg0 + 2 >= T and extras:
                        for k in sorted(extras):
                            extras.pop(k)()
                    for i, t in enumerate(tl):
                        nc.tensor.matmul(
                            ctx2[:, :],
                            lhsT=VP[:, t, 2 * P:3 * P],
                            rhs=es2[:, i * CW:(i + 1) * CW],
                            start=(t == 0), stop=(t == T - 1),
                        )

                cs2 = norm.tile([DH, CW], F32, tag="cs2", name=f"cs2_{ci}")
                nc.vector.tensor_copy(cs2, ctx2[0:DH, :])
                nc.vector.tensor_copy(dn[:, 2 * CW:3 * CW], ctx2[DH:DH + 1, :])
                nc.vector.reciprocal_approx_fast(rc[:, 2 * CW:3 * CW], dn[:, 2 * CW:3 * CW])
                bc0 = norm.tile([DH, CW], F32, tag="bc0", name=f"bc0_{ci}")
                bc1 = norm.tile([DH, CW], F32, tag="bc1", name=f"bc1_{ci}")
                bc2 = norm.tile([DH, CW], F32, tag="bc2", name=f"bc2_{ci}")
                nc.gpsimd.partition_broadcast(bc0, rc[:, 0:CW])
                nc.vector.tensor_mul(CTX01[0:DH, c0:c0 + CW], cs0, bc0)
                nc.gpsimd.partition_broadcast(bc1, rc[:, CW:2 * CW])
                nc.vector.tensor_mul(CTX01[DH:P, c0:c0 + CW], cs1, bc1)
                nc.gpsimd.partition_broadcast(bc2, rc[:, 2 * CW:3 * CW])
                nc.vector.tensor_mul(CTX2[:, c0:c0 + CW], cs2, bc2)
                # Q projection for the next chunk runs on PE while DVE/gpsimd
                # finish this chunk's normalization.
                if ci == 0 and NCH > 1:
                    qproj(CW, 0)
                    qproj(CW, 1)

            # tail: O-projection of the last chunk
            for m in range((NCH - 1) * 4, NCH * 4):
                oproj(m)
    nc.compile()
    return nc


def _get_prog(KP):
    if KP not in _prog_cache:
        _prog_cache[KP] = _build_nc(KP)
    return _prog_cache[KP]


def _fmt_w(w):
    # [768, 192] -> [128, 6*192]: row kt*128+p, col j  ->  [p, kt*192+j]
    import ml_dtypes
    return np.ascontiguousarray(
        w.reshape(NKT, P, DQ).transpose(1, 0, 2).reshape(P, NKT * DQ)
    ).astype(ml_dtypes.bfloat16)


def _run(inputs, trace=False):
    import ml_dtypes
    from concourse.bass_utils import run_bass_kernel_spmd

    BF = ml_dtypes.bfloat16

    query = np.asarray(inputs["query"], dtype=np.float32)
    key = np.asarray(inputs["key"], dtype=np.float32)
    value = np.asarray(inputs["value"], dtype=np.float32)
    mask = np.asarray(inputs["mask"])
    Wq = np.asarray(inputs["Wq"], dtype=np.float32)
    bq = np.asarray(inputs["bq"], dtype=np.float32)
    Wk = np.asarray(inputs["Wk"], dtype=np.float32)
    bk = np.asarray(inputs["bk"], dtype=np.float32)
    Wv = np.asarray(inputs["Wv"], dtype=np.float32)
    bv = np.asarray(inputs["bv"], dtype=np.float32)
    Wo = np.asarray(inputs["Wo"], dtype=np.float32)
    bo = np.asarray(inputs["bo"], dtype=np.float32)

    idx = [np.nonzero(mask[b, 0, 0] != 0)[0] for b in range(B)]
    keff = [len(i) for i in idx]
    KP = max(P, ((max(keff) + P - 1) // P) * P)
    T = KP // P

    nc = _get_prog(KP)

    per_batch = {}
    for b in range(B):
        xqT = np.ascontiguousarray(query[b].T).astype(BF)
        xkT = np.zeros((DM, KP), dtype=BF)
        xkT[:, :keff[b]] = key[b][idx[b]].T.astype(BF)
        xvT = np.zeros((DM, KP), dtype=BF)
        xvT[:, :keff[b]] = value[b][idx[b]].T.astype(BF)
        vmf = np.zeros((KP,), dtype=np.float32)
        vmf[:keff[b]] = 1.0
        vm2 = np.ascontiguousarray(vmf.reshape(T, P).T)  # [128, T]
        # packed layouts: one large contiguous DMA each
        xqfm = np.ascontiguousarray(
            xqT.reshape(NKT, P, NCH, CW).transpose(1, 2, 0, 3).reshape(P, -1))
        xkfm = np.ascontiguousarray(
            xkT.reshape(NKT, P, KP).transpose(1, 0, 2).reshape(P, -1))
        xvfm = np.ascontiguousarray(
            xvT.reshape(NKT, P, KP).transpose(1, 0, 2).reshape(P, -1))
        per_batch[b] = (xqfm, xkfm, xvfm, vm2)

    in_maps = []
    for core in range(NCORES):
        b, g = core // G, core % G
        xqfm, xkfm, xvfm, vm2 = per_batch[b]
        sl = slice(g * DQ, (g + 1) * DQ)
        wo_sl = Wo[sl, :]
        wofm = np.zeros((P, 2 * DM), dtype=BF)
        wofm[0:P, 0:DM] = wo_sl[0:P, :].astype(BF)
        wofm[0:DH, DM:2 * DM] = wo_sl[P:DQ, :].astype(BF)
        consm = np.zeros((P, 4 + T + DQ), dtype=np.float32)
        consm[0:P, 0] = bq[sl][0:P]
        consm[0:DH, 1] = bq[sl][P:DQ]
        consm[0:P, 2] = bk[sl][0:P]
        consm[0:DH, 3] = bk[sl][P:DQ]
        consm[:, 4:4 + T] = vm2
        consm[:, 4 + T:4 + T + DQ] = bv[sl].reshape(1, DQ)
        in_maps.append({
            "xqf": xqfm,
            "xkf": xkfm,
            "xvf": xvfm,
            "wq": _fmt_w(Wq[:, sl]),
            "wk": _fmt_w(Wk[:, sl]),
            "wv": _fmt_w(Wv[:, sl]),
            "wof": wofm,
            "cons": consm,
        })

    res = run_bass_kernel_spmd(nc, in_maps, list(range(NCORES)), trace=trace)

    outp = np.zeros((B, S, DM), dtype=np.float32)
    for core in range(NCORES):
        outp[core // G] += res.results[core]["out"].astype(np.float32)
    outp += bo.reshape(1, 1, DM)
    return outp, res


def kernel(**inputs) -> np.ndarray:
    out, _ = _run(inputs, trace=False)
    return out


if __name__ == "__main__":
    nc = _build_nc(1152)
    print("build OK")
